# revision 2
# baseline (speedup 1.0000x reference)
"""Kronecker layer forward on 8 TRN2 NeuronCores.

Computes y = gelu_exact(x @ kron(B, A)) + bias for
  x [16384, 4096] f32, A [64, 64], B [64, 64], bias [4096].

Math: with x3 = x.reshape(n, 64, 64) (feature f = i*64 + k),
  y[b, j*64+l] = sum_{i,k} x3[b,i,k] * B[i,j] * A[k,l].

Per supertile s we pick 4 tokens t(g,h) (g,h in {0,1}) and form one
128x128 SBUF tile
  xt[(g,i), (h,k)] = x[t(g,h), i*64+k]
then chain two TensorE matmuls with the DATA as the stationary operand
and a block-diagonal factor as the moving one:
  o1 = xt.T @ blockdiag(B,B)    -> o1[(h,k), (g,j)]   (contract (g,i))
  o2 = u.T  @ blockdiag(A,A)    -> o2[(g,j), (h,l)]   (contract (h,k))
(u = o1 copied to SBUF). o2 is exactly the y-layout view
y[t(g,h), j*64+l] — both contraction dims land on partitions with zero
transposes.

Everything on-chip is bf16 (tolerance is 2e-2; bf16 lands ~2e-3):
 - x is converted to bf16 on the host, halving input HBM traffic;
 - bf16 matmuls stream 1 cycle/row at 128-wide moving operands (fp32r
   needed a 256-wide concat with a garbage half to hit that rate);
 - FWL (fast weight load) is compiler-automatic for 128-col non-fp32
   stationaries, halving the LDWEIGHTS cost of the data tiles;
 - y is stored bf16 and upconverted on the host, halving output traffic.

The host pre-permutes x to [blk, g, i, r, k] (r = 2s+h, token
t = g*tpc/2 + blk*2NB + r) so each block is ONE fully contiguous 1MB
DMA whose element order matches the SBUF tile [p=(g,i), f=(r,k)]
exactly. y is written the same way ([blk, g, j, r, l]) and
inverse-permuted on the host.

Sharding: pure data-parallel over the token dim — 2048 tokens per core,
A/B/bias replicated, no collectives.
"""

import numpy as np

N_CORES = 8
TOKENS = 16384
D = 4096
TPC = TOKENS // N_CORES  # tokens per core

_CACHE = {}


def _build_bf16(tpc, with_bias, n_cores):
    import concourse.bacc as bacc
    import concourse.mybir as mybir
    import concourse.tile as tile

    f32 = mybir.dt.float32
    bf16 = mybir.dt.bfloat16

    nsuper = tpc // 4
    NB = min(32, nsuper)          # supertiles per block (1MB bf16 DMA)
    assert nsuper % NB == 0
    nblocks = nsuper // NB
    GRP = 8                       # supertiles per PSUM pack (2 banks)
    assert NB % GRP == 0

    nc = bacc.Bacc(
        "TRN2",
        target_bir_lowering=False,
        debug=False,
        num_devices=n_cores,
    )
    x_d = nc.dram_tensor(
        "x", [nblocks, 128 * 2 * NB * 64], bf16, kind="ExternalInput"
    ).ap()
    bmat_d = nc.dram_tensor("bd", [128, 128], bf16, kind="ExternalInput").ap()
    amat_d = nc.dram_tensor("ad", [128, 128], bf16, kind="ExternalInput").ap()
    if with_bias:
        bias_d = nc.dram_tensor("bias_t", [128, 128], f32, kind="ExternalInput").ap()
    y_d = nc.dram_tensor(
        "y", [nblocks, 128 * 2 * NB * 64], bf16, kind="ExternalOutput"
    ).ap()

    with tile.TileContext(nc) as tc:
        with (
            tc.tile_pool(name="const", bufs=1) as constp,
            tc.tile_pool(name="xp", bufs=3) as xp,
            tc.tile_pool(name="up", bufs=4) as up,
            tc.tile_pool(name="yp", bufs=3) as yp,
            tc.tile_pool(name="ps1", bufs=2, space="PSUM") as ps1,
            tc.tile_pool(name="ps2", bufs=2, space="PSUM") as ps2,
        ):
            bmat = constp.tile([128, 128], bf16)
            nc.sync.dma_start(bmat[:], bmat_d)
            amat = constp.tile([128, 128], bf16)
            nc.sync.dma_start(amat[:], amat_d)
            if with_bias:
                bias_t = constp.tile([128, 128], f32)
                nc.sync.dma_start(bias_t[:], bias_d)

            for blk in range(nblocks):
                xbig = xp.tile([128, NB * 128], bf16)
                ybig = yp.tile([128, NB * 128], bf16)
                nc.sync.dma_start(xbig[:], x_d[blk])

                for grp in range(NB // GRP):
                    o1 = ps1.tile([128, GRP * 128], f32)
                    o2 = ps2.tile([128, GRP * 128], f32)
                    u = up.tile([128, GRP * 128], bf16)
                    for q in range(GRP):
                        s = grp * GRP + q
                        nc.tensor.matmul(
                            o1[:, q * 128 : (q + 1) * 128],
                            xbig[:, s * 128 : (s + 1) * 128],
                            bmat[:],
                        )
                    nc.vector.tensor_copy(u[:], o1[:])
                    for q in range(GRP):
                        nc.tensor.matmul(
                            o2[:, q * 128 : (q + 1) * 128],
                            u[:, q * 128 : (q + 1) * 128],
                            amat[:],
                        )
                    ydst = ybig[:, grp * GRP * 128 : (grp + 1) * GRP * 128]
                    nc.scalar.activation(
                        ydst, o2[:], mybir.ActivationFunctionType.Gelu
                    )
                    if with_bias:
                        bseg = ydst.rearrange("p (q f) -> p q f", f=128)
                        bsrc = bias_t[:].unsqueeze(1).broadcast_to([128, GRP, 128])
                        nc.vector.tensor_add(bseg, bseg, bsrc)

                nc.scalar.dma_start(y_d[blk], ybig[:])

    nc.compile()
    return nc


def _get_nc(tpc, mm_impl, with_bias, n_cores=N_CORES):
    key = (tpc, mm_impl, with_bias, n_cores)
    if key not in _CACHE:
        assert mm_impl == "bf16"
        _CACHE[key] = _build_bf16(tpc, with_bias, n_cores)
    return _CACHE[key]


def _make_weights(A, B):
    import ml_dtypes

    Bd = np.zeros((128, 128), np.float32)
    Bd[:64, :64] = B
    Bd[64:, 64:] = B
    Ad = np.zeros((128, 128), np.float32)
    Ad[:64, :64] = A
    Ad[64:, 64:] = A
    return {
        "bd": Bd.astype(ml_dtypes.bfloat16),
        "ad": Ad.astype(ml_dtypes.bfloat16),
    }


def _run(x, A, B, bias, mm_impl="bf16", tpc=TPC, trace=False):
    import ml_dtypes
    from concourse.bass_utils import run_bass_kernel_spmd

    bf16 = ml_dtypes.bfloat16
    n = x.shape[0]
    n_cores = n // tpc
    assert n == n_cores * tpc

    with_bias = bool(np.any(bias))
    nc = _get_nc(tpc, mm_impl, with_bias, n_cores)
    wmaps = _make_weights(np.asarray(A, np.float32), np.asarray(B, np.float32))

    nsuper = tpc // 4
    NB = min(32, nsuper)
    nblocks = nsuper // NB

    def permute_x(xs):
        # tokens x feats -> [g, blk, r, i, k] -> [blk, g, i, r, k]
        v = xs.reshape(2, nblocks, 2 * NB, 64, 64).transpose(1, 0, 3, 2, 4)
        return np.ascontiguousarray(v).astype(bf16).reshape(nblocks, -1)

    def unpermute_y(yd):
        # [blk, g, j, r, l] -> [g, blk, r, j, l] -> tokens x feats
        v = yd.reshape(nblocks, 2, 64, 2 * NB, 64).transpose(1, 0, 3, 2, 4)
        return np.ascontiguousarray(v).astype(np.float32).reshape(tpc, D)

    in_maps = []
    for c in range(n_cores):
        m = {"x": permute_x(np.asarray(x[c * tpc : (c + 1) * tpc], dtype=np.float32))}
        m.update(wmaps)
        if with_bias:
            m["bias_t"] = np.ascontiguousarray(
                np.tile(bias.astype(np.float32).reshape(64, 64), (2, 2))
            )
        in_maps.append(m)

    res = run_bass_kernel_spmd(
        nc, in_maps, list(range(n_cores)), trace=trace,
        trace_cores=list(range(n_cores)) if trace else None,
    )
    y = np.concatenate([unpermute_y(np.asarray(r["y"])) for r in res.results], axis=0)
    return y.astype(np.float32), res


def kernel(x, A, B, bias):
    y, _ = _run(
        np.asarray(x), np.asarray(A), np.asarray(B), np.asarray(bias),
        mm_impl="bf16",
    )
    return y


# revision 4
# speedup vs baseline: 1.0391x; 1.0391x over previous
"""Kronecker layer forward on 8 TRN2 NeuronCores.

Computes y = gelu_exact(x @ kron(B, A)) + bias for
  x [16384, 4096] f32, A [64, 64], B [64, 64], bias [4096].

Math: with x3 = x.reshape(n, 64, 64) (feature f = i*64 + k),
  y[b, j*64+l] = sum_{i,k} x3[b,i,k] * B[i,j] * A[k,l].

Per supertile s we pick 4 tokens t(g,h) (g,h in {0,1}) and form one
128x128 SBUF tile
  xt[(g,i), (h,k)] = x[t(g,h), i*64+k]
then chain two TensorE matmuls with the DATA as the stationary operand
and a block-diagonal factor as the moving one:
  o1 = xt.T @ blockdiag(B,B)    -> o1[(h,k), (g,j)]   (contract (g,i))
  o2 = u.T  @ blockdiag(A,A)    -> o2[(g,j), (h,l)]   (contract (h,k))
(u = o1 copied to SBUF). o2 is exactly the y-layout view
y[t(g,h), j*64+l] — both contraction dims land on partitions with zero
transposes.

Everything on-chip is bf16 (tolerance is 2e-2; bf16 lands ~2e-3):
 - x is converted to bf16 on the host, halving input HBM traffic;
 - bf16 matmuls stream 1 cycle/row at 128-wide moving operands (fp32r
   needed a 256-wide concat with a garbage half to hit that rate);
 - FWL (fast weight load) is compiler-automatic for 128-col non-fp32
   stationaries, halving the LDWEIGHTS cost of the data tiles;
 - y is stored bf16 and upconverted on the host, halving output traffic.

The host pre-permutes x to [blk, g, i, r, k] (r = 2s+h, token
t = g*tpc/2 + blk*2NB + r) so each block is ONE fully contiguous 1MB
DMA whose element order matches the SBUF tile [p=(g,i), f=(r,k)]
exactly. y is written the same way ([blk, g, j, r, l]) and
inverse-permuted on the host.

Sharding: pure data-parallel over the token dim — 2048 tokens per core,
A/B/bias replicated, no collectives.
"""

import numpy as np

N_CORES = 8
TOKENS = 16384
D = 4096
TPC = TOKENS // N_CORES  # tokens per core

_CACHE = {}


def _block_schedule(nsuper):
    """Supertiles per block: small first block to prime the pipeline,
    1MB (32-supertile) blocks in steady state, tapered tail so the final
    load->compute->store chain is short."""
    sizes = []
    rem = nsuper
    if rem >= 32:
        sizes.append(16)
        rem -= 16
    while rem > 16:
        take = min(32, rem - 16)
        sizes.append(take)
        rem -= take
    for s in (8, 4, 4):
        if rem >= s:
            sizes.append(s)
            rem -= s
    if rem:
        sizes.append(rem)
    assert sum(sizes) == nsuper and all(s % 4 == 0 for s in sizes)
    return sizes


def _build_bf16(tpc, with_bias, n_cores):
    import concourse.bacc as bacc
    import concourse.mybir as mybir
    import concourse.tile as tile

    f32 = mybir.dt.float32
    bf16 = mybir.dt.bfloat16

    nsuper = tpc // 4
    sizes = _block_schedule(nsuper)
    NBMAX = max(sizes)

    nc = bacc.Bacc(
        "TRN2",
        target_bir_lowering=False,
        debug=False,
        num_devices=n_cores,
    )
    x_d = nc.dram_tensor("x", [tpc * 4096], bf16, kind="ExternalInput").ap()
    bmat_d = nc.dram_tensor("bd", [128, 128], bf16, kind="ExternalInput").ap()
    amat_d = nc.dram_tensor("ad", [128, 128], bf16, kind="ExternalInput").ap()
    if with_bias:
        bias_d = nc.dram_tensor("bias_t", [128, 128], f32, kind="ExternalInput").ap()
    y_d = nc.dram_tensor("y", [tpc * 4096], bf16, kind="ExternalOutput").ap()

    with tile.TileContext(nc) as tc:
        with (
            tc.tile_pool(name="const", bufs=1) as constp,
            tc.tile_pool(name="xp", bufs=4) as xp,
            tc.tile_pool(name="up", bufs=4) as up,
            tc.tile_pool(name="yp", bufs=4) as yp,
            tc.tile_pool(name="ps1", bufs=2, space="PSUM") as ps1,
            tc.tile_pool(name="ps2", bufs=2, space="PSUM") as ps2,
        ):
            # weights via the (otherwise idle) gpsimd queue so the first
            # x-block load is the sync queue's first DMA
            bmat = constp.tile([128, 128], bf16)
            nc.gpsimd.dma_start(bmat[:], bmat_d)
            amat = constp.tile([128, 128], bf16)
            nc.gpsimd.dma_start(amat[:], amat_d)
            if with_bias:
                bias_t = constp.tile([128, 128], f32)
                nc.gpsimd.dma_start(bias_t[:], bias_d)

            off = 0
            for NB in sizes:
                nel = 128 * 2 * NB * 64
                xbig = xp.tile([128, NBMAX * 128], bf16)
                ybig = yp.tile([128, NBMAX * 128], bf16)
                nc.sync.dma_start(xbig[:, : NB * 128], x_d[off : off + nel])

                GRP = min(8, NB)
                for grp in range(NB // GRP):
                    o1 = ps1.tile([128, 8 * 128], f32)
                    o2 = ps2.tile([128, 8 * 128], f32)
                    u = up.tile([128, 8 * 128], bf16)
                    for q in range(GRP):
                        s = grp * GRP + q
                        nc.tensor.matmul(
                            o1[:, q * 128 : (q + 1) * 128],
                            xbig[:, s * 128 : (s + 1) * 128],
                            bmat[:],
                        )
                    nc.vector.tensor_copy(
                        u[:, : GRP * 128], o1[:, : GRP * 128]
                    )
                    for q in range(GRP):
                        nc.tensor.matmul(
                            o2[:, q * 128 : (q + 1) * 128],
                            u[:, q * 128 : (q + 1) * 128],
                            amat[:],
                        )
                    ydst = ybig[:, grp * GRP * 128 : (grp + 1) * GRP * 128]
                    nc.scalar.activation(
                        ydst, o2[:, : GRP * 128],
                        mybir.ActivationFunctionType.Gelu,
                    )
                    if with_bias:
                        bseg = ydst.rearrange("p (q f) -> p q f", f=128)
                        bsrc = bias_t[:].unsqueeze(1).broadcast_to([128, GRP, 128])
                        nc.vector.tensor_add(bseg, bseg, bsrc)

                nc.scalar.dma_start(y_d[off : off + nel], ybig[:, : NB * 128])
                off += nel

    nc.compile()
    return nc


def _get_nc(tpc, mm_impl, with_bias, n_cores=N_CORES):
    key = (tpc, mm_impl, with_bias, n_cores)
    if key not in _CACHE:
        assert mm_impl == "bf16"
        _CACHE[key] = _build_bf16(tpc, with_bias, n_cores)
    return _CACHE[key]


def _make_weights(A, B):
    import ml_dtypes

    Bd = np.zeros((128, 128), np.float32)
    Bd[:64, :64] = B
    Bd[64:, 64:] = B
    Ad = np.zeros((128, 128), np.float32)
    Ad[:64, :64] = A
    Ad[64:, 64:] = A
    return {
        "bd": Bd.astype(ml_dtypes.bfloat16),
        "ad": Ad.astype(ml_dtypes.bfloat16),
    }


def _run(x, A, B, bias, mm_impl="bf16", tpc=TPC, trace=False):
    import ml_dtypes
    from concourse.bass_utils import run_bass_kernel_spmd

    bf16 = ml_dtypes.bfloat16
    n = x.shape[0]
    n_cores = n // tpc
    assert n == n_cores * tpc

    with_bias = bool(np.any(bias))
    nc = _get_nc(tpc, mm_impl, with_bias, n_cores)
    wmaps = _make_weights(np.asarray(A, np.float32), np.asarray(B, np.float32))

    nsuper = tpc // 4
    sizes = _block_schedule(nsuper)

    def permute_x(xs):
        # tokens x feats -> per block [g, i, r, k] slabs, concatenated flat
        v = xs.reshape(2, tpc // 2, 64, 64).astype(bf16)  # [g, r_tot, i, k]
        slabs = []
        r0 = 0
        for NB in sizes:
            blk = v[:, r0 : r0 + 2 * NB]               # [g, r, i, k]
            slabs.append(np.ascontiguousarray(blk.transpose(0, 2, 1, 3)).ravel())
            r0 += 2 * NB
        return np.concatenate(slabs)

    def unpermute_y(yd):
        # per block [g, j, r, l] slabs -> tokens x feats
        out = np.empty((2, tpc // 2, 64, 64), np.float32)  # [g, r_tot, j, l]
        e0, r0 = 0, 0
        for NB in sizes:
            nel = 128 * 2 * NB * 64
            blk = yd[e0 : e0 + nel].reshape(2, 64, 2 * NB, 64)
            out[:, r0 : r0 + 2 * NB] = blk.transpose(0, 2, 1, 3).astype(np.float32)
            e0 += nel
            r0 += 2 * NB
        return out.reshape(tpc, D)

    in_maps = []
    for c in range(n_cores):
        m = {"x": permute_x(np.asarray(x[c * tpc : (c + 1) * tpc], dtype=np.float32))}
        m.update(wmaps)
        if with_bias:
            m["bias_t"] = np.ascontiguousarray(
                np.tile(bias.astype(np.float32).reshape(64, 64), (2, 2))
            )
        in_maps.append(m)

    res = run_bass_kernel_spmd(
        nc, in_maps, list(range(n_cores)), trace=trace,
        trace_cores=list(range(n_cores)) if trace else None,
    )
    y = np.concatenate([unpermute_y(np.asarray(r["y"])) for r in res.results], axis=0)
    return y.astype(np.float32), res


def kernel(x, A, B, bias):
    y, _ = _run(
        np.asarray(x), np.asarray(A), np.asarray(B), np.asarray(bias),
        mm_impl="bf16",
    )
    return y


# revision 5
# speedup vs baseline: 1.0560x; 1.0163x over previous
"""Kronecker layer forward on 8 TRN2 NeuronCores.

Computes y = gelu_exact(x @ kron(B, A)) + bias for
  x [16384, 4096] f32, A [64, 64], B [64, 64], bias [4096].

Math: with x3 = x.reshape(n, 64, 64) (feature f = i*64 + k),
  y[b, j*64+l] = sum_{i,k} x3[b,i,k] * B[i,j] * A[k,l].

Per supertile s we pick 4 tokens t(g,h) (g,h in {0,1}) and form one
128x128 SBUF tile
  xt[(g,i), (h,k)] = x[t(g,h), i*64+k]
then chain two TensorE matmuls with the DATA as the stationary operand
and a block-diagonal factor as the moving one:
  o1 = xt.T @ blockdiag(B,B)    -> o1[(h,k), (g,j)]   (contract (g,i))
  o2 = u.T  @ blockdiag(A,A)    -> o2[(g,j), (h,l)]   (contract (h,k))
(u = o1 copied to SBUF). o2 is exactly the y-layout view
y[t(g,h), j*64+l] — both contraction dims land on partitions with zero
transposes.

Everything on-chip is bf16 (tolerance is 2e-2; this lands 3.7e-3):
 - x is converted to bf16 on the host, halving input HBM traffic;
 - bf16 matmuls stream 1 cycle/row at 128-wide moving operands (fp32r
   needed a 256-wide concat with a garbage half to hit that rate);
 - FWL (fast weight load) is compiler-automatic for 128-col non-fp32
   stationaries, halving the LDWEIGHTS cost of the data tiles;
 - y is stored bf16 and upconverted on the host, halving output traffic.

The host pre-permutes x to per-block [g, i, r, k] slabs (r = 2s+h,
token t = g*tpc/2 + 2*block_off + r) so each block is ONE fully
contiguous DMA whose element order matches the SBUF tile
[p=(g,i), f=(r,k)] exactly. y is written the same way ([g, j, r, l]
slabs) and inverse-permuted on the host. Block sizes taper
(16, 32...32, 8, 4, 4 supertiles): a small first block primes the
pipeline and the small last blocks shrink the serial
load->compute->store tail to ~2us.

Sharding: pure data-parallel over the token dim — 2048 tokens per core,
A/B/bias replicated, no collectives.

Measured (8-core SPMD, per-core): HBM-DMA-bound at 355-362 GB/s of the
~358 GB/s per-core ceiling; DMA window ~94us (= 33.6MB / 358GB/s
roofline), plus ~7us fixed framework preamble and ~9us semaphore-reset
teardown. PE ~56us, DVE (psum->sbuf bf16 cast) ~68us, ACT (gelu)
~71us, all hidden under DMA. HW exec ~106-117us vs 235.5us baseline.
Dead ends investigated: int8/fp8 input DMA (fp8 rel err 2.7e-2 fails
the 2e-2 gate; int8 passes at 1.3e-2 but bass matmul has no int8 and
GpSimd converts at only ~65 G elem/s = 130us), For_i hardware loops to
shrink the teardown (~2us/back-edge all-engine barrier eats the win).
"""

import numpy as np

N_CORES = 8
TOKENS = 16384
D = 4096
TPC = TOKENS // N_CORES  # tokens per core

_CACHE = {}


def _block_schedule(nsuper):
    """Supertiles per block: small first block to prime the pipeline,
    1MB (32-supertile) blocks in steady state, tapered tail so the final
    load->compute->store chain is short."""
    sizes = []
    rem = nsuper
    if rem >= 32:
        sizes.append(16)
        rem -= 16
    while rem > 16:
        take = min(32, rem - 16)
        sizes.append(take)
        rem -= take
    for s in (8, 4, 4):
        if rem >= s:
            sizes.append(s)
            rem -= s
    if rem:
        sizes.append(rem)
    assert sum(sizes) == nsuper and all(s % 4 == 0 for s in sizes)
    return sizes


def _build_bf16(tpc, with_bias, n_cores):
    import concourse.bacc as bacc
    import concourse.mybir as mybir
    import concourse.tile as tile

    f32 = mybir.dt.float32
    bf16 = mybir.dt.bfloat16

    nsuper = tpc // 4
    sizes = _block_schedule(nsuper)
    NBMAX = max(sizes)

    nc = bacc.Bacc(
        "TRN2",
        target_bir_lowering=False,
        debug=False,
        num_devices=n_cores,
    )
    x_d = nc.dram_tensor("x", [tpc * 4096], bf16, kind="ExternalInput").ap()
    bmat_d = nc.dram_tensor("bd", [128, 128], bf16, kind="ExternalInput").ap()
    amat_d = nc.dram_tensor("ad", [128, 128], bf16, kind="ExternalInput").ap()
    if with_bias:
        bias_d = nc.dram_tensor("bias_t", [128, 128], f32, kind="ExternalInput").ap()
    y_d = nc.dram_tensor("y", [tpc * 4096], bf16, kind="ExternalOutput").ap()

    with tile.TileContext(nc) as tc:
        with (
            tc.tile_pool(name="const", bufs=1) as constp,
            tc.tile_pool(name="xp", bufs=4) as xp,
            tc.tile_pool(name="up", bufs=4) as up,
            tc.tile_pool(name="yp", bufs=4) as yp,
            tc.tile_pool(name="ps1", bufs=2, space="PSUM") as ps1,
            tc.tile_pool(name="ps2", bufs=2, space="PSUM") as ps2,
        ):
            # weights via the (otherwise idle) gpsimd queue so the first
            # x-block load is the sync queue's first DMA
            bmat = constp.tile([128, 128], bf16)
            nc.gpsimd.dma_start(bmat[:], bmat_d)
            amat = constp.tile([128, 128], bf16)
            nc.gpsimd.dma_start(amat[:], amat_d)
            if with_bias:
                bias_t = constp.tile([128, 128], f32)
                nc.gpsimd.dma_start(bias_t[:], bias_d)

            off = 0
            for NB in sizes:
                nel = 128 * 2 * NB * 64
                xbig = xp.tile([128, NBMAX * 128], bf16)
                ybig = yp.tile([128, NBMAX * 128], bf16)
                nc.sync.dma_start(xbig[:, : NB * 128], x_d[off : off + nel])

                GRP = min(8, NB)
                for grp in range(NB // GRP):
                    o1 = ps1.tile([128, 8 * 128], f32)
                    o2 = ps2.tile([128, 8 * 128], f32)
                    u = up.tile([128, 8 * 128], bf16)
                    for q in range(GRP):
                        s = grp * GRP + q
                        nc.tensor.matmul(
                            o1[:, q * 128 : (q + 1) * 128],
                            xbig[:, s * 128 : (s + 1) * 128],
                            bmat[:],
                        )
                    nc.vector.tensor_copy(
                        u[:, : GRP * 128], o1[:, : GRP * 128]
                    )
                    for q in range(GRP):
                        nc.tensor.matmul(
                            o2[:, q * 128 : (q + 1) * 128],
                            u[:, q * 128 : (q + 1) * 128],
                            amat[:],
                        )
                    ydst = ybig[:, grp * GRP * 128 : (grp + 1) * GRP * 128]
                    nc.scalar.activation(
                        ydst, o2[:, : GRP * 128],
                        mybir.ActivationFunctionType.Gelu,
                    )
                    if with_bias:
                        bseg = ydst.rearrange("p (q f) -> p q f", f=128)
                        bsrc = bias_t[:].unsqueeze(1).broadcast_to([128, GRP, 128])
                        nc.vector.tensor_add(bseg, bseg, bsrc)

                nc.scalar.dma_start(y_d[off : off + nel], ybig[:, : NB * 128])
                off += nel

    nc.compile()
    return nc


def _get_nc(tpc, mm_impl, with_bias, n_cores=N_CORES):
    key = (tpc, mm_impl, with_bias, n_cores)
    if key not in _CACHE:
        assert mm_impl == "bf16"
        _CACHE[key] = _build_bf16(tpc, with_bias, n_cores)
    return _CACHE[key]


def _make_weights(A, B):
    import ml_dtypes

    Bd = np.zeros((128, 128), np.float32)
    Bd[:64, :64] = B
    Bd[64:, 64:] = B
    Ad = np.zeros((128, 128), np.float32)
    Ad[:64, :64] = A
    Ad[64:, 64:] = A
    return {
        "bd": Bd.astype(ml_dtypes.bfloat16),
        "ad": Ad.astype(ml_dtypes.bfloat16),
    }


def _run(x, A, B, bias, mm_impl="bf16", tpc=TPC, trace=False):
    import ml_dtypes
    from concourse.bass_utils import run_bass_kernel_spmd

    bf16 = ml_dtypes.bfloat16
    n = x.shape[0]
    n_cores = n // tpc
    assert n == n_cores * tpc

    with_bias = bool(np.any(bias))
    nc = _get_nc(tpc, mm_impl, with_bias, n_cores)
    wmaps = _make_weights(np.asarray(A, np.float32), np.asarray(B, np.float32))

    nsuper = tpc // 4
    sizes = _block_schedule(nsuper)

    def permute_x(xs):
        # tokens x feats -> per block [g, i, r, k] slabs, concatenated flat
        v = xs.reshape(2, tpc // 2, 64, 64).astype(bf16)  # [g, r_tot, i, k]
        slabs = []
        r0 = 0
        for NB in sizes:
            blk = v[:, r0 : r0 + 2 * NB]               # [g, r, i, k]
            slabs.append(np.ascontiguousarray(blk.transpose(0, 2, 1, 3)).ravel())
            r0 += 2 * NB
        return np.concatenate(slabs)

    def unpermute_y(yd):
        # per block [g, j, r, l] slabs -> tokens x feats
        out = np.empty((2, tpc // 2, 64, 64), np.float32)  # [g, r_tot, j, l]
        e0, r0 = 0, 0
        for NB in sizes:
            nel = 128 * 2 * NB * 64
            blk = yd[e0 : e0 + nel].reshape(2, 64, 2 * NB, 64)
            out[:, r0 : r0 + 2 * NB] = blk.transpose(0, 2, 1, 3).astype(np.float32)
            e0 += nel
            r0 += 2 * NB
        return out.reshape(tpc, D)

    in_maps = []
    for c in range(n_cores):
        m = {"x": permute_x(np.asarray(x[c * tpc : (c + 1) * tpc], dtype=np.float32))}
        m.update(wmaps)
        if with_bias:
            m["bias_t"] = np.ascontiguousarray(
                np.tile(bias.astype(np.float32).reshape(64, 64), (2, 2))
            )
        in_maps.append(m)

    res = run_bass_kernel_spmd(
        nc, in_maps, list(range(n_cores)), trace=trace,
        trace_cores=list(range(n_cores)) if trace else None,
    )
    y = np.concatenate([unpermute_y(np.asarray(r["y"])) for r in res.results], axis=0)
    return y.astype(np.float32), res


def kernel(x, A, B, bias):
    y, _ = _run(
        np.asarray(x), np.asarray(A), np.asarray(B), np.asarray(bias),
        mm_impl="bf16",
    )
    return y


# revision 6
# speedup vs baseline: 1.1949x; 1.1315x over previous
"""Kronecker layer forward on 8 TRN2 NeuronCores.

Computes y = gelu_exact(x @ kron(B, A)) + bias for
  x [16384, 4096] f32, A [64, 64], B [64, 64], bias [4096].

Math: with x3 = x.reshape(n, 64, 64) (feature f = i*64 + k),
  y[b, j*64+l] = sum_{i,k} x3[b,i,k] * B[i,j] * A[k,l].

Per supertile s we pick 4 tokens t(g,h) (g,h in {0,1}) and form one
128x128 SBUF tile
  xt[(g,i), (h,k)] = x[t(g,h), i*64+k]
then chain two TensorE matmuls with the DATA as the stationary operand
and a block-diagonal factor as the moving one:
  o1 = xt.T @ blockdiag(B,B)    -> o1[(h,k), (g,j)]   (contract (g,i))
  o2 = u.T  @ blockdiag(A,A)    -> o2[(g,j), (h,l)]   (contract (h,k))
(u = o1 copied to SBUF). o2 is exactly the y-layout view
y[t(g,h), j*64+l] — both contraction dims land on partitions with zero
transposes.

Everything on-chip is bf16 (tolerance is 2e-2; this lands 3.7e-3):
 - x is converted to bf16 on the host, halving input HBM traffic;
 - bf16 matmuls stream 1 cycle/row at 128-wide moving operands (fp32r
   needed a 256-wide concat with a garbage half to hit that rate);
 - FWL (fast weight load) is compiler-automatic for 128-col non-fp32
   stationaries, halving the LDWEIGHTS cost of the data tiles;
 - y is stored bf16 and upconverted on the host, halving output traffic.

The host pre-permutes x to per-block [g, i, r, k] slabs (r = 2s+h,
token t = g*tpc/2 + 2*block_off + r) so each block is ONE fully
contiguous DMA whose element order matches the SBUF tile
[p=(g,i), f=(r,k)] exactly. y is written the same way ([g, j, r, l]
slabs) and inverse-permuted on the host. Block sizes taper
(16, 32...32, 8, 4, 4 supertiles): a small first block primes the
pipeline and the small last blocks shrink the serial
load->compute->store tail to ~2us.

Sharding: pure data-parallel over the token dim — 2048 tokens per core,
A/B/bias replicated, no collectives.

Measured (8-core SPMD, per-core): HBM-DMA-bound at 355-362 GB/s of the
~358 GB/s per-core ceiling; DMA window ~94us (= 33.6MB / 358GB/s
roofline), plus ~7us fixed framework preamble and ~9us semaphore-reset
teardown. PE ~56us, DVE (psum->sbuf bf16 cast) ~68us, ACT (gelu)
~71us, all hidden under DMA. HW exec ~106-117us vs 235.5us baseline.
Dead ends investigated: int8/fp8 input DMA (fp8 rel err 2.7e-2 fails
the 2e-2 gate; int8 passes at 1.3e-2 but bass matmul has no int8 and
GpSimd converts at only ~65 G elem/s = 130us), For_i hardware loops to
shrink the teardown (~2us/back-edge all-engine barrier eats the win).
"""

import numpy as np

N_CORES = 8
TOKENS = 16384
D = 4096
TPC = TOKENS // N_CORES  # tokens per core

_CACHE = {}


def _block_schedule(nsuper):
    """Supertiles per block: small first block to prime the pipeline,
    1MB (32-supertile) blocks in steady state, tapered tail so the final
    load->compute->store chain is short."""
    sizes = []
    rem = nsuper
    if rem >= 32:
        sizes.append(16)
        rem -= 16
    while rem > 16:
        take = min(32, rem - 16)
        sizes.append(take)
        rem -= take
    for s in (8, 4, 4):
        if rem >= s:
            sizes.append(s)
            rem -= s
    if rem:
        sizes.append(rem)
    assert sum(sizes) == nsuper and all(s % 4 == 0 for s in sizes)
    return sizes


def _build_bf16(tpc, with_bias, n_cores):
    import concourse.bacc as bacc
    import concourse.mybir as mybir
    import concourse.tile as tile

    f32 = mybir.dt.float32
    bf16 = mybir.dt.bfloat16

    nsuper = tpc // 4
    sizes = _block_schedule(nsuper)
    NBMAX = max(sizes)

    nc = bacc.Bacc(
        "TRN2",
        target_bir_lowering=False,
        debug=False,
        num_devices=n_cores,
    )
    x_d = nc.dram_tensor("x", [tpc * 4096], bf16, kind="ExternalInput").ap()
    bmat_d = nc.dram_tensor("bd", [128, 128], bf16, kind="ExternalInput").ap()
    amat_d = nc.dram_tensor("ad", [128, 128], bf16, kind="ExternalInput").ap()
    if with_bias:
        bias_d = nc.dram_tensor("bias_t", [128, 128], f32, kind="ExternalInput").ap()
    y_d = nc.dram_tensor("y", [tpc * 4096], bf16, kind="ExternalOutput").ap()

    with tile.TileContext(nc) as tc:
        with (
            tc.tile_pool(name="const", bufs=1) as constp,
            tc.tile_pool(name="xp", bufs=4) as xp,
            tc.tile_pool(name="up", bufs=4) as up,
            tc.tile_pool(name="yp", bufs=4) as yp,
            tc.tile_pool(name="ps1", bufs=2, space="PSUM") as ps1,
            tc.tile_pool(name="ps2", bufs=2, space="PSUM") as ps2,
        ):
            # weights via the (otherwise idle) gpsimd queue so the first
            # x-block load is the sync queue's first DMA
            bmat = constp.tile([128, 128], bf16)
            nc.gpsimd.dma_start(bmat[:], bmat_d)
            amat = constp.tile([128, 128], bf16)
            nc.gpsimd.dma_start(amat[:], amat_d)
            if with_bias:
                bias_t = constp.tile([128, 128], f32)
                nc.gpsimd.dma_start(bias_t[:], bias_d)

            off = 0
            for NB in sizes:
                nel = 128 * 2 * NB * 64
                xbig = xp.tile([128, NBMAX * 128], bf16)
                ybig = yp.tile([128, NBMAX * 128], bf16)
                nc.sync.dma_start(xbig[:, : NB * 128], x_d[off : off + nel])

                GRP = min(8, NB)
                for grp in range(NB // GRP):
                    o1 = ps1.tile([128, 8 * 128], f32)
                    o2 = ps2.tile([128, 8 * 128], f32)
                    u = up.tile([128, 8 * 128], bf16)
                    for q in range(GRP):
                        s = grp * GRP + q
                        nc.tensor.matmul(
                            o1[:, q * 128 : (q + 1) * 128],
                            xbig[:, s * 128 : (s + 1) * 128],
                            bmat[:],
                        )
                    nc.vector.tensor_copy(
                        u[:, : GRP * 128], o1[:, : GRP * 128]
                    )
                    for q in range(GRP):
                        nc.tensor.matmul(
                            o2[:, q * 128 : (q + 1) * 128],
                            u[:, q * 128 : (q + 1) * 128],
                            amat[:],
                        )
                    ydst = ybig[:, grp * GRP * 128 : (grp + 1) * GRP * 128]
                    nc.scalar.activation(
                        ydst, o2[:, : GRP * 128],
                        mybir.ActivationFunctionType.Gelu,
                    )
                    if with_bias:
                        bseg = ydst.rearrange("p (q f) -> p q f", f=128)
                        bsrc = bias_t[:].unsqueeze(1).broadcast_to([128, GRP, 128])
                        nc.vector.tensor_add(bseg, bseg, bsrc)

                nc.scalar.dma_start(y_d[off : off + nel], ybig[:, : NB * 128])
                off += nel

    nc.compile()
    return nc


def _get_nc(tpc, mm_impl, with_bias, n_cores=N_CORES):
    key = (tpc, mm_impl, with_bias, n_cores)
    if key not in _CACHE:
        assert mm_impl == "bf16"
        _CACHE[key] = _build_bf16(tpc, with_bias, n_cores)
    return _CACHE[key]


def _make_weights(A, B):
    import ml_dtypes

    Bd = np.zeros((128, 128), np.float32)
    Bd[:64, :64] = B
    Bd[64:, 64:] = B
    Ad = np.zeros((128, 128), np.float32)
    Ad[:64, :64] = A
    Ad[64:, 64:] = A
    return {
        "bd": Bd.astype(ml_dtypes.bfloat16),
        "ad": Ad.astype(ml_dtypes.bfloat16),
    }


def _run(x, A, B, bias, mm_impl="bf16", tpc=TPC, trace=False, trace_one=False):
    import ml_dtypes
    from concourse.bass_utils import run_bass_kernel_spmd

    bf16 = ml_dtypes.bfloat16
    n = x.shape[0]
    n_cores = n // tpc
    assert n == n_cores * tpc

    with_bias = bool(np.any(bias))
    nc = _get_nc(tpc, mm_impl, with_bias, n_cores)
    wmaps = _make_weights(np.asarray(A, np.float32), np.asarray(B, np.float32))

    nsuper = tpc // 4
    sizes = _block_schedule(nsuper)

    def permute_x(xs):
        # tokens x feats -> per block [g, i, r, k] slabs, concatenated flat
        v = xs.reshape(2, tpc // 2, 64, 64).astype(bf16)  # [g, r_tot, i, k]
        slabs = []
        r0 = 0
        for NB in sizes:
            blk = v[:, r0 : r0 + 2 * NB]               # [g, r, i, k]
            slabs.append(np.ascontiguousarray(blk.transpose(0, 2, 1, 3)).ravel())
            r0 += 2 * NB
        return np.concatenate(slabs)

    def unpermute_y(yd):
        # per block [g, j, r, l] slabs -> tokens x feats
        out = np.empty((2, tpc // 2, 64, 64), np.float32)  # [g, r_tot, j, l]
        e0, r0 = 0, 0
        for NB in sizes:
            nel = 128 * 2 * NB * 64
            blk = yd[e0 : e0 + nel].reshape(2, 64, 2 * NB, 64)
            out[:, r0 : r0 + 2 * NB] = blk.transpose(0, 2, 1, 3).astype(np.float32)
            e0 += nel
            r0 += 2 * NB
        return out.reshape(tpc, D)

    in_maps = []
    for c in range(n_cores):
        m = {"x": permute_x(np.asarray(x[c * tpc : (c + 1) * tpc], dtype=np.float32))}
        m.update(wmaps)
        if with_bias:
            m["bias_t"] = np.ascontiguousarray(
                np.tile(bias.astype(np.float32).reshape(64, 64), (2, 2))
            )
        in_maps.append(m)

    res = run_bass_kernel_spmd(
        nc, in_maps, list(range(n_cores)), trace=trace or trace_one,
        trace_cores=[0] if trace_one else (list(range(n_cores)) if trace else None),
    )
    y = np.concatenate([unpermute_y(np.asarray(r["y"])) for r in res.results], axis=0)
    return y.astype(np.float32), res


def kernel(x, A, B, bias):
    y, _ = _run(
        np.asarray(x), np.asarray(A), np.asarray(B), np.asarray(bias),
        mm_impl="bf16",
    )
    return y
